# revision 11
# baseline (speedup 1.0000x reference)
"""Self-contained Trainium2 Bass kernel: ChildSum TreeLSTM forest encoder.

Forest of B=4 full 4-ary trees, depth 8 (87381 nodes/tree), E=H=128.
Sharding: 8 cores, each owns half a tree (the 2 subtrees rooted at two of the
root's four children = 43690 nodes). Levels 0..7 run on-device with no
cross-core communication; the single root node per tree is combined on host.

Device layout: transposed [H=128 partitions, nodes free]. Host pre-transposes
x per core and permutes each level's nodes into child-major order so every
child gather on device is a unit-stride slice. x-path matmuls run in float32r
(1 col/cycle), recurrent-path matmuls in bf16 (h stored bf16, c stored f32).
"""

import numpy as np

try:
    import concourse.bass as bass
except ImportError:  # pragma: no cover - env fallback
    import sys

    for _p in (
        "/opt/trn_rl_repo",
        "/root/.axon_site/_ro/trn_rl_repo",
        "/root/.axon_site/_ro/pypackages",
        "/root/.axon_site",
    ):
        if _p not in sys.path:
            sys.path.append(_p)
    import concourse.bass as bass

import ml_dtypes
from contextlib import ExitStack

import concourse.tile as tile
from concourse import mybir
from concourse.bass_utils import run_bass_kernel_spmd

# ---- problem geometry (hardcoded) ----
B, E, H, D, BR = 4, 128, 128, 8, 4
LEVEL_SIZES = [BR ** (D - l) for l in range(D + 1)]  # leaves ... root
OFFSETS = [0]
for _n in LEVEL_SIZES:
    OFFSETS.append(OFFSETS[-1] + _n)
N_NODES = OFFSETS[-1]  # 87381

NCORES = 8
NL = [2 * 4 ** (7 - l) for l in range(8)]  # per-core level sizes 32768..2
LOFF = [0]
for _n in NL:
    LOFF.append(LOFF[-1] + _n)
NCOLS = LOFF[-1]  # 43690

CH = 512  # matmul/ACT chunk (one PSUM bank of fp32)
SC = 16  # leaf super-chunks (leaf+L1 fusion granularity)

F32 = mybir.dt.float32
F32R = mybir.dt.float32r
BF16 = mybir.dt.bfloat16
SIG = mybir.ActivationFunctionType.Sigmoid
TANH = mybir.ActivationFunctionType.Tanh


def _split_excess_waits(nc, limit=1):
    """Walrus codegen only accepts `limit` sem-waits per instruction; hoist
    extras into preceding same-engine NoOps."""
    ctr = 0
    for bb in nc.m.functions[0].blocks:
        new_insts = []
        for inst in bb.instructions:
            si = inst.sync_info
            if si is not None and si.on_wait and len(si.on_wait) > limit:
                waits = list(si.on_wait)
                extra, keep = waits[:-limit], waits[-limit:]
                for i in range(0, len(extra), limit):
                    ctr += 1
                    new_insts.append(
                        mybir.InstNoOp(
                            name=f"wait-split-{ctr}",
                            engine=inst.engine,
                            ins=[],
                            outs=[],
                            sync_info=mybir.SyncInfo(
                                on_wait=extra[i : i + limit], on_update=[]
                            ),
                        )
                    )
                inst.sync_info = mybir.SyncInfo(
                    on_wait=keep, on_update=list(si.on_update or [])
                )
            new_insts.append(inst)
        bb.instructions[:] = new_insts
    return ctr


def _build_program(zero_bias: bool):
    nc = bass.Bass("TRN2", target_bir_lowering=False, debug=False)
    xt_d = nc.dram_tensor("xt", [128, NCOLS], F32R, kind="ExternalInput")
    wx_d = nc.dram_tensor("wx", [128, 512], F32R, kind="ExternalInput")
    uiou_d = nc.dram_tensor("uiou", [128, 384], F32R, kind="ExternalInput")
    uf_d = nc.dram_tensor("uf", [128, 128], F32R, kind="ExternalInput")
    b_d = nc.dram_tensor("bias", [128, 4], F32, kind="ExternalInput")
    out_d = nc.dram_tensor("out", [128, 4], F32, kind="ExternalOutput")

    with tile.TileContext(nc) as tc, ExitStack() as es:
        wp = es.enter_context(tc.tile_pool(name="w", bufs=1))
        store = es.enter_context(tc.tile_pool(name="store", bufs=1))
        leafp = es.enter_context(tc.tile_pool(name="leafsc", bufs=2))
        xp = es.enter_context(tc.tile_pool(name="x", bufs=2))
        gp = es.enter_context(tc.tile_pool(name="g", bufs=2))
        mp = es.enter_context(tc.tile_pool(name="m", bufs=2))
        pp = es.enter_context(tc.tile_pool(name="ps", bufs=8, space="PSUM"))

        # weights
        wx = wp.tile([128, 512], F32R, tag="wx")
        uiou = wp.tile([128, 384], F32R, tag="uiou")
        uf = wp.tile([128, 128], F32R, tag="uf")
        bias = wp.tile([128, 4], F32, tag="bias")
        nc.sync.dma_start(wx[:], wx_d.ap())
        nc.sync.dma_start(uiou[:], uiou_d.ap())
        nc.sync.dma_start(uf[:], uf_d.ap())
        nc.sync.dma_start(bias[:], b_d.ap())
        b_i, b_f, b_o, b_u = (bias[:, g : g + 1] for g in range(4))

        # persistent per-level stores (levels 1..7): h in f32r (matmul-ready), c in f32
        h_st = {}
        c_st = {}
        for l in range(1, 8):
            h_st[l] = store.tile([128, NL[l]], F32R, tag=f"h{l}", name=f"h_st{l}")
            c_st[l] = store.tile([128, NL[l]], F32, tag=f"c{l}", name=f"c_st{l}")

        WXI, WXF, WXO, WXU = (wx[:, g * 128 : (g + 1) * 128] for g in range(4))
        UI, UO, UU = (uiou[:, g * 128 : (g + 1) * 128] for g in range(3))

        xt_leaf3d = xt_d.ap()[:, 0 : 4 * NL[1]].rearrange("p (k c) -> p k c", k=4)

        def leaf_pair(sc, kA, h0_t, c0_t):
            """Two leaf child-block chunks (kA, kA+1) of super-chunk sc: 1024 leaves."""
            xt_t = xp.tile([128, 1024], F32R, tag="xleaf", bufs=3)
            nc.sync.dma_start(
                xt_t[:].rearrange("p (k c) -> p k c", k=2),
                xt_leaf3d[:, kA : kA + 2, sc * CH : (sc + 1) * CH],
            )
            xh = (xt_t[:, 0:512], xt_t[:, 512:1024])

            gi = gp.tile([128, 1024], F32, tag="gio")
            go = gp.tile([128, 1024], F32, tag="gf01")
            gu = gp.tile([128, 1024], F32, tag="gf23")
            for half in range(2):
                sl = slice(half * 512, half * 512 + 512)
                for W, bb, fn, gt in ((WXI, b_i, SIG, gi), (WXO, b_o, SIG, go), (WXU, b_u, TANH, gu)):
                    ps = pp.tile([128, 512], F32, tag="ps1", name="ps")
                    nc.tensor.matmul(ps[:], W, xh[half], start=True, stop=True)
                    nc.scalar.activation(gt[:, sl], ps[:], fn, bias=bb)

            csl = c0_t[:, kA : kA + 2, :].rearrange("p a b -> p (a b)")
            hsl = h0_t[:, kA : kA + 2, :].rearrange("p a b -> p (a b)")
            tct = gp.tile([128, 1024], F32, tag="tct")
            for half in range(2):
                sl = slice(half * 512, half * 512 + 512)
                nc.gpsimd.tensor_mul(csl[:, sl], gi[:, sl], gu[:, sl])
                nc.scalar.activation(tct[:, sl], csl[:, sl], TANH)
                nc.vector.tensor_mul(hsl[:, sl], go[:, sl], tct[:, sl])

        def internal_chunk(l, q0, n, hprev, cprev):
            """One chunk of n nodes at storage cols [q0, q0+n) of level l>=1.

            hprev(k)/cprev(k): APs of the k-th child slice (f32r / f32)."""
            xt_t = xp.tile([128, CH], F32R, tag="xint")
            c0 = LOFF[l] + q0
            nc.sync.dma_start(xt_t[:, :n], xt_d.ap()[:, c0 : c0 + n])
            xv = xt_t[:, :n]

            hs = mp.tile([128, CH], F32R, tag="hs")
            nc.vector.tensor_add(hs[:, :n], hprev(0), hprev(1))
            nc.vector.tensor_add(hs[:, :n], hs[:, :n], hprev(2))
            nc.vector.tensor_add(hs[:, :n], hs[:, :n], hprev(3))
            hsv = hs[:, :n]

            gio = gp.tile([128, 1024], F32, tag="gio")
            f01 = gp.tile([128, 1024], F32, tag="gf01")
            f23 = gp.tile([128, 1024], F32, tag="gf23")
            gu = gp.tile([128, 512], F32, tag="gu")

            def gate(W, U, rhs2, out_sl, fn, bb):
                ps = pp.tile([128, 512], F32, tag="ps1", name="ps")
                nc.tensor.matmul(ps[:, 0:n], W, xv, start=True, stop=False)
                nc.tensor.matmul(ps[:, 0:n], U, rhs2, start=False, stop=True)
                nc.scalar.activation(out_sl, ps[:, 0:n], fn, bias=bb)

            gate(WXI, UI, hsv, gio[:, 0:n], SIG, b_i)
            gate(WXO, UO, hsv, gio[:, n : 2 * n], SIG, b_o)
            for k in range(4):
                ft = f01 if k < 2 else f23
                s = (k % 2) * n
                gate(WXF, uf[:], hprev(k), ft[:, s : s + n], SIG, b_f)
            gate(WXU, UU, hsv, gu[:, 0:n], TANH, b_u)

            m0 = mp.tile([128, CH], F32, tag="m0")
            m1 = mp.tile([128, CH], F32, tag="m1")
            fc = mp.tile([128, CH], F32, tag="fc")
            f_sl = lambda k: (f01 if k < 2 else f23)[:, (k % 2) * n : (k % 2) * n + n]
            nc.gpsimd.tensor_mul(m0[:, :n], f_sl(0), cprev(0))
            nc.gpsimd.tensor_mul(m1[:, :n], f_sl(1), cprev(1))
            nc.vector.tensor_add(fc[:, :n], m0[:, :n], m1[:, :n])
            nc.vector.tensor_mul(m0[:, :n], f_sl(2), cprev(2))
            nc.vector.tensor_add(fc[:, :n], fc[:, :n], m0[:, :n])
            nc.vector.tensor_mul(m1[:, :n], f_sl(3), cprev(3))
            nc.vector.tensor_add(fc[:, :n], fc[:, :n], m1[:, :n])

            tct = gp.tile([128, 1024], F32, tag="tct")
            iu = tct[:, 512 : 512 + n]
            nc.vector.tensor_mul(iu, gio[:, 0:n], gu[:, 0:n])
            csl = c_st[l][:, q0 : q0 + n]
            nc.vector.tensor_add(csl, iu, fc[:, :n])
            nc.scalar.activation(tct[:, :n], csl, TANH)
            nc.vector.tensor_mul(h_st[l][:, q0 : q0 + n], gio[:, n : 2 * n], tct[:, :n])

        # ---- levels 0+1 fused in super-chunks ----
        for sc in range(SC):
            h0_t = leafp.tile([128, 4, CH], F32R, tag="h0")
            c0_t = leafp.tile([128, 4, CH], F32, tag="c0")
            leaf_pair(sc, 0, h0_t, c0_t)
            leaf_pair(sc, 2, h0_t, c0_t)
            internal_chunk(
                1,
                sc * CH,
                CH,
                hprev=lambda k: h0_t[:, k, :],
                cprev=lambda k: c0_t[:, k, :],
            )

        # ---- levels 2..7 ----
        for l in range(2, 8):
            nl = NL[l]
            for q0 in range(0, nl, CH):
                n = min(CH, nl - q0)
                internal_chunk(
                    l,
                    q0,
                    n,
                    hprev=lambda k, l=l, q0=q0, n=n: h_st[l - 1][:, k * NL[l] + q0 : k * NL[l] + q0 + n],
                    cprev=lambda k, l=l, q0=q0, n=n: c_st[l - 1][:, k * NL[l] + q0 : k * NL[l] + q0 + n],
                )

        # ---- outputs: h7|c7 -> [128, 4] f32 ----
        out_t = wp.tile([128, 4], F32, tag="out")
        nc.vector.tensor_copy(out_t[:, 0:2], h_st[7][:])
        nc.vector.tensor_copy(out_t[:, 2:4], c_st[7][:])
        nc.sync.dma_start(out_d.ap(), out_t[:])

    _split_excess_waits(nc)
    return nc


_PROGRAMS = {}


def _get_program(zero_bias: bool):
    key = bool(zero_bias)
    if key not in _PROGRAMS:
        _PROGRAMS[key] = _build_program(key)
    return _PROGRAMS[key]


def _orders():
    """Per-level child-major storage permutations (within-core natural index)."""
    ords = [None] * 8
    o = np.arange(2, dtype=np.int64)
    ords[7] = o
    for l in range(6, -1, -1):
        o = np.concatenate([4 * ords[l + 1] + k for k in range(4)])
        ords[l] = o
    return ords


def make_in_maps(x, Wx, Uiou, Uf, b):
    """Host-side shard/permute/transpose. Returns per-core input dicts."""
    x = np.asarray(x, dtype=np.float32)
    Wx = np.ascontiguousarray(np.asarray(Wx, dtype=np.float32))
    Uiou = np.asarray(Uiou, dtype=np.float32)
    Uf = np.asarray(Uf, dtype=np.float32)
    b = np.asarray(b, dtype=np.float32)

    ords = _orders()
    uiou_c = np.ascontiguousarray(Uiou)
    uf_c = np.ascontiguousarray(Uf)
    bias_pg = np.ascontiguousarray(b.reshape(4, 128).T)  # [p, gate]

    in_maps = []
    for c in range(NCORES):
        tb, s = divmod(c, 2)
        xt = np.empty((128, NCOLS), np.float32)
        for l in range(8):
            nl = NL[l]
            xs = x[tb, OFFSETS[l] + s * nl : OFFSETS[l] + (s + 1) * nl, :]
            xt[:, LOFF[l] : LOFF[l] + nl] = xs[ords[l]].T
        in_maps.append(
            {"xt": xt, "wx": Wx, "uiou": uiou_c, "uf": uf_c, "bias": bias_pg}
        )
    return in_maps


def finish_on_host(outs, x, Wx, Uiou, Uf, b):
    """Root-level (4 nodes) combine from per-core [128, 4] outputs."""

    def sig(z):
        return 1.0 / (1.0 + np.exp(-z))

    x = np.asarray(x)
    Wx64 = np.asarray(Wx, np.float64)
    Uiou64 = np.asarray(Uiou, np.float64)
    Uf64 = np.asarray(Uf, np.float64)
    b64 = np.asarray(b, np.float64)

    hc = np.empty((B, 4, H), np.float64)
    cc = np.empty((B, 4, H), np.float64)
    for tb in range(B):
        for s in range(2):
            o = np.asarray(outs[2 * tb + s], np.float64)  # [128, 4]
            hc[tb, 2 * s : 2 * s + 2] = o[:, 0:2].T
            cc[tb, 2 * s : 2 * s + 2] = o[:, 2:4].T

    xr = np.asarray(x[:, OFFSETS[8], :], np.float64)  # [B, 128] root x
    g = xr @ Wx64 + b64
    xi, xf, xo, xu = np.split(g, 4, axis=1)
    hsum = hc.sum(1)
    hi, ho, hu = np.split(hsum @ Uiou64, 3, axis=1)
    i = sig(xi + hi)
    o_ = sig(xo + ho)
    u = np.tanh(xu + hu)
    f = sig(xf[:, None, :] + hc @ Uf64)
    c = i * u + (f * cc).sum(1)
    h = o_ * np.tanh(c)
    return h.astype(np.float32), c.astype(np.float32)


def kernel(x, Wx, Uiou, Uf, b):
    x = np.asarray(x, dtype=np.float32)
    Wx = np.asarray(Wx, dtype=np.float32)
    Uiou = np.asarray(Uiou, dtype=np.float32)
    Uf = np.asarray(Uf, dtype=np.float32)
    b = np.asarray(b, dtype=np.float32)

    in_maps = make_in_maps(x, Wx, Uiou, Uf, b)
    nc = _get_program(zero_bias=not np.any(b))
    res = run_bass_kernel_spmd(nc, in_maps, list(range(NCORES)))
    outs = [res.results[c]["out"] for c in range(NCORES)]
    return finish_on_host(outs, x, Wx, Uiou, Uf, b)
